# revision 20
# baseline (speedup 1.0000x reference)
"""CommNet (B=4096, A=50, DIN=128, H=256, DOUT=64, K=2) on 8 TRN2 NeuronCores.

Data-parallel over batch: 512 examples (25600 agent-tokens) per core, weights
replicated, feature-major ([feature, token]) on-chip layout with the host
pre-transposing each x shard (fp16 - halves input DMA bytes).

Final design (266 us HW exec, rel err 6.1e-4; baseline was 286 us):
- All dense matmuls fp16 at N=400 (one 8-example sub-tile per PSUM bank);
  N=400 streams at 1 col/cycle back-to-back, where N=512 measured 0.61 ns/col.
- tanh ACTIVATEs cover whole 3-bank PSUM groups ({3,3,2} sub-tiles) so the
  ScalarE ~172-cycle per-op overhead amortizes; groups are emitted
  m-interleaved so the 2-slot PSUM ring never backs up into the PE queue.
- Agent-mean path: S = per-example sums (one DVE reduce per [128,3200] half),
  pcw = (512/A*W_bot)^T S on the PE duplicated into both partition halves by
  column tiling, u16 = fp16(pcw) on DVE, then broadcast back over agents by
  ROW-TILED pairs of fp16 matmuls against a constant 0/2^-9 selector - the
  two members of a pair contract in rows 0-63 / 64-127 of the PE array and
  run concurrently (145 ns/matmul vs 224 serial).
- Decoder (M=64) is COLUMN-TILED: pairs of sub-tiles run concurrently in the
  two 64-column halves of the array (133 ns avg), outputs land in the two
  partition halves of shared banks, copied to fp16 and DMA'd as strided
  even/odd token blocks. Decoder bias is folded into the host-side unshard.
- Schedule: 2 groups of 4 supertiles; cw matmuls staggered one supertile
  behind their producing reduce, decoders staggered into the last comm pass
  (DVE is idle there), and the next group's x DMAs prefetched a full group
  early - each fix removes a measured multi-us in-order engine-queue stall.
"""

import numpy as np

import concourse.bacc as bacc
import concourse.bass as bass
import concourse.tile as tile
from concourse import mybir
from concourse.bass_utils import run_bass_kernel_spmd

N_CORES = 8
B, A, DIN, H, DOUT, K = 4096, 50, 128, 256, 64, 2
BS = B // N_CORES          # examples per core
TOK = BS * A               # tokens per core
ST_EX = 64                 # examples per supertile
ST = ST_EX * A             # 3200 tokens per supertile
BANK = 512                 # fp32 elems per PSUM bank
SUB = 400                  # tokens per sub-tile (8 examples; N=400 streams
                           # at 1 col/cycle on HW where N=512 does not)
WBOT_SCALE = 512.0         # host folds 512/A into w_bot; sel carries 2^-9
DEC_COLTILE = True

# (token_lo, [widths]) groups; each group = one 3-bank PSUM tile filled,
# activated, and released before the next group needs its ring slot.
GROUPS = [(0, [400, 400, 400]), (1200, [400, 400, 400]), (2400, [400, 400])]

F32 = mybir.dt.float32
F16 = mybir.dt.float16
Tanh = mybir.ActivationFunctionType.Tanh


def build_nc(n_supertiles=BS // ST_EX, ilv=4):
    tok = n_supertiles * ST
    nc = bacc.Bacc(
        "TRN2",
        target_bir_lowering=False,
        debug=False,
        enable_asserts=True,
        num_devices=N_CORES,
    )
    xT = nc.dram_tensor("xT", [DIN, tok], F16, kind="ExternalInput")
    w_enc = nc.dram_tensor("w_enc", [DIN, H], F16, kind="ExternalInput")
    b_enc = nc.dram_tensor("b_enc", [128, 2], F32, kind="ExternalInput")
    w_top = nc.dram_tensor("w_top", [128, K * 2 * H], F16, kind="ExternalInput")
    w_bot = nc.dram_tensor("w_bot", [128, K * 2 * H], F16, kind="ExternalInput")
    b_h = nc.dram_tensor("b_h", [128, K * 2], F32, kind="ExternalInput")
    w_dec = nc.dram_tensor("w_dec", [128, 2 * DOUT], F16, kind="ExternalInput")
    sel = nc.dram_tensor("sel", [128, ST], F16, kind="ExternalInput")
    y = nc.dram_tensor("y", [DOUT, tok], F16, kind="ExternalOutput")

    with tile.TileContext(nc) as tc:
        with (
            tc.tile_pool(name="wpool", bufs=1) as wpool,
            tc.tile_pool(name="xpool", bufs=2 * ilv) as xpool,
            tc.tile_pool(name="hpool", bufs=ilv) as hpool,
            tc.tile_pool(name="spool", bufs=ilv) as spool,
            tc.tile_pool(name="opool", bufs=2) as opool,
            tc.tile_pool(name="psmm", bufs=2, space=bass.MemorySpace.PSUM) as psmm,
            tc.tile_pool(name="pscw", bufs=2, space=bass.MemorySpace.PSUM) as pscw,
        ):
            wenc_sb = wpool.tile([DIN, H], F16)
            nc.sync.dma_start(wenc_sb[:], w_enc[:])
            benc_sb = wpool.tile([128, 2], F32)
            nc.sync.dma_start(benc_sb[:], b_enc[:])
            wtop_sb = wpool.tile([128, K * 2 * H], F16)
            wbot_sb = wpool.tile([128, K * 2 * H], F16)
            bh_sb = wpool.tile([128, K * 2], F32)
            nc.sync.dma_start(bh_sb[:], b_h[:])
            wdec_sb = wpool.tile([128, 2 * DOUT], F16)
            sel_sb = wpool.tile([128, ST], F16)

            def load_bulk_weights():
                nc.gpsimd.dma_start(wtop_sb[:], w_top[:])
                nc.gpsimd.dma_start(wbot_sb[:], w_bot[:])
                nc.gpsimd.dma_start(wdec_sb[:], w_dec[:])
                nc.gpsimd.dma_start(sel_sb[:], sel[:])

            def make_state(s):
                xt = xpool.tile([DIN, ST], F16, tag="xt", name=f"xt_{s}")
                for c0 in range(0, ST, 1600):
                    nc.gpsimd.dma_start(
                        xt[:, c0 : c0 + 1600],
                        xT[:, s * ST + c0 : s * ST + c0 + 1600],
                    )
                hA = hpool.tile([128, 2, ST], F16, tag="hA", name=f"hA_{s}")
                hB = hpool.tile([128, 2, ST], F16, tag="hB", name=f"hB_{s}")
                S_t = [
                    spool.tile([128, 2, ST_EX], F16, tag=f"S{k}", name=f"S{k}_{s}")
                    for k in range(K)
                ]
                u_t = [
                    spool.tile([128, H], F16, tag=f"u{k}", name=f"u{k}_{s}")
                    for k in range(K)
                ]
                return {"s": s, "xt": xt, "hA": hA, "hB": hB, "S": S_t, "u": u_t}

            def reduce_S(st, h, k, m):
                seg = h[:, m, :].rearrange("p (e a) -> p e a", a=A)
                with nc.allow_low_precision(reason="fp16 agent-sum"):
                    nc.vector.reduce_sum(
                        st["S"][k][:, m, :], seg, axis=mybir.AxisListType.X
                    )

            def cw_phase(st, k):
                s = st["s"]
                pcw = pscw.tile([128, BANK], F32, tag="pcw", name=f"pcw_{s}_{k}")
                for kc in range(2):
                    off = (k * 2 + kc) * H
                    for half in range(2):
                        p0 = half * ST_EX
                        nc.tensor.matmul(
                            pcw[p0 : p0 + ST_EX, 0:H],
                            st["S"][k][:, kc, :],
                            wbot_sb[:, off : off + H],
                            start=(kc == 0),
                            stop=(kc == 1),
                            skip_group_check=True,
                        )
                with nc.allow_low_precision(reason="fp16 u (pcw is 512x scaled)"):
                    nc.vector.tensor_copy(st["u"][k][:], pcw[:, 0:H])

            def act_group(ps, h_out, m, lo, widths, bias):
                span = sum(widths)
                n = len(widths)
                w = widths[0]
                src = ps[:].rearrange("p (g b) -> p g b", b=BANK)[:, 0:n, 0:w]
                dst = h_out[:, m, lo : lo + span].rearrange(
                    "p (g b) -> p g b", b=w
                )
                nc.scalar.activation(dst, src, Tanh, bias=bias)

            def enc_phase(st):
                s, xt, hA = st["s"], st["xt"], st["hA"]
                for lo, widths in GROUPS:
                    for m in range(2):
                        ps = psmm.tile(
                            [128, 3 * BANK], F32, tag="ps",
                            name=f"pse_{s}_{m}_{lo}",
                        )
                        for j, w in enumerate(widths):
                            t0 = lo + j * SUB
                            nc.tensor.matmul(
                                ps[:, j * BANK : j * BANK + w],
                                wenc_sb[:, m * 128 : (m + 1) * 128],
                                xt[:, t0 : t0 + w],
                                start=True,
                                stop=True,
                            )
                        act_group(ps, hA, m, lo, widths, benc_sb[:, m : m + 1])
                        if lo == GROUPS[-1][0]:
                            reduce_S(st, hA, 0, m)
                cw_phase(st, 0)

            def comm_phase(st, k):
                s = st["s"]
                hcur = st["hA"] if k == 0 else st["hB"]
                hnxt = st["hB"] if k == 0 else st["hA"]
                u16 = st["u"][k]
                for lo, widths in GROUPS:
                    for m in range(2):
                        n = len(widths)
                        ps = psmm.tile(
                            [128, 3 * BANK], F32, tag="ps",
                            name=f"psc_{s}_{k}_{lo}_{m}",
                        )
                        for kc in range(2):
                            off = (k * 2 + kc) * H + m * 128
                            for j, w in enumerate(widths):
                                t0 = lo + j * SUB
                                nc.tensor.matmul(
                                    ps[:, j * BANK : j * BANK + w],
                                    wtop_sb[:, off : off + 128],
                                    hcur[:, kc, t0 : t0 + w],
                                    start=(kc == 0),
                                    stop=False,
                                )
                        j = 0
                        while j < n:
                            for half in range(1 + (j + 1 < n)):
                                jj = j + half
                                p0 = half * ST_EX
                                t0 = lo + jj * SUB
                                nc.tensor.matmul(
                                    ps[:, jj * BANK : jj * BANK + widths[jj]],
                                    u16[p0 : p0 + ST_EX,
                                        m * 128 : (m + 1) * 128],
                                    sel_sb[p0 : p0 + ST_EX,
                                           t0 : t0 + widths[jj]],
                                    start=False,
                                    stop=True,
                                    skip_group_check=True,
                                )
                            j += 2
                        act_group(
                            ps, hnxt, m, lo, widths,
                            bh_sb[:, k * 2 + m : k * 2 + m + 1],
                        )
                        if k + 1 < K and lo == GROUPS[-1][0]:
                            reduce_S(st, hnxt, k + 1, m)

            def dec_phase(st):
                s = st["s"]
                hcur = st["hA"] if K % 2 == 0 else st["hB"]
                # 8 sub-tiles as 4 column-tiled pairs: the two members of a
                # pair run concurrently in the two 64-column halves of the
                # PE array (out partitions 0-63 / 64-127), one bank each.
                out_t = opool.tile([128, 4 * SUB], F16, tag="out",
                                   name=f"out_{s}")
                ps = psmm.tile([128, 3 * BANK], F32, tag="ps", name=f"psd_{s}")
                ps2 = psmm.tile([128, 3 * BANK], F32, tag="ps", name=f"psd2_{s}")
                for kc in range(2):
                    wslice = wdec_sb[:, kc * DOUT : (kc + 1) * DOUT]
                    for pair in range(4):
                        pt, bank = (ps, pair) if pair < 3 else (ps2, 0)
                        for half in range(2):
                            t0 = (2 * pair + half) * SUB
                            p0 = half * DOUT
                            nc.tensor.matmul(
                                pt[p0 : p0 + DOUT,
                                   bank * BANK : bank * BANK + SUB],
                                wslice,
                                hcur[:, kc, t0 : t0 + SUB],
                                start=(kc == 0),
                                stop=(kc == 1),
                                skip_group_check=True,
                            )
                with nc.allow_low_precision(reason="fp16 output"):
                    for half in range(2):
                        p0, p1 = half * DOUT, (half + 1) * DOUT
                        nc.vector.tensor_copy(
                            out_t[p0:p1, 0:1200].rearrange(
                                "p (g b) -> p g b", b=SUB
                            ),
                            ps[p0:p1].rearrange("p (g b) -> p g b", b=BANK)[
                                :, 0:3, 0:SUB
                            ],
                        )
                        nc.vector.tensor_copy(
                            out_t[p0:p1, 1200:1600],
                            ps2[p0:p1, 0:SUB],
                        )
                base = s * ST
                for half in range(2):
                    p0, p1 = half * DOUT, (half + 1) * DOUT
                    yv = y[:, base : base + ST].rearrange(
                        "p (g b) -> p g b", b=2 * SUB
                    )[:, :, half * SUB : (half + 1) * SUB]
                    sv = out_t[p0:p1, 0:1600].rearrange(
                        "p (g b) -> p g b", b=SUB
                    )
                    nc.sync.dma_start(yv, sv)

            assert n_supertiles % ilv == 0 or n_supertiles < ilv
            step = min(ilv, n_supertiles)
            sgroups = [
                list(range(s0, s0 + step))
                for s0 in range(0, n_supertiles, step)
            ]
            sts = [make_state(s) for s in sgroups[0]]
            load_bulk_weights()
            for st in sts:
                enc_phase(st)
            nxt = [make_state(s) for s in sgroups[1]] if len(sgroups) > 1 else None
            for gi, grp in enumerate(sgroups):
                for k in range(K):
                    for i, st in enumerate(sts):
                        comm_phase(st, k)
                        if k + 1 < K and i >= 1:
                            cw_phase(sts[i - 1], k + 1)
                        if k == K - 1 and i >= 1:
                            # stagger decoders into the last comm pass: their
                            # DVE copies land while DVE is otherwise idle and
                            # the next comm phase fills any PE wait.
                            dec_phase(sts[i - 1])
                    if k + 1 < K:
                        cw_phase(sts[-1], k + 1)
                dec_phase(sts[-1])
                if nxt is not None:
                    for nst in nxt:
                        enc_phase(nst)
                    sts = nxt
                    nxt = (
                        [make_state(s) for s in sgroups[gi + 2]]
                        if gi + 2 < len(sgroups)
                        else None
                    )

    nc.compile()
    return nc


def host_inputs(x, W_enc, b_enc, W_h, b_h, W_dec, b_dec, n_cores=N_CORES, bs=BS):
    """Shard x over cores (pre-transposed, fp16); replicate weights."""
    x = np.asarray(x, np.float32)
    f16 = np.float16
    W_h = np.asarray(W_h, np.float32)
    common = {
        "w_enc": np.ascontiguousarray(np.asarray(W_enc, np.float32)).astype(f16),
        "b_enc": np.ascontiguousarray(
            np.asarray(b_enc, np.float32).reshape(2, 128).T
        ),
        "w_top": np.ascontiguousarray(
            W_h[:, :H, :].reshape(K * 2, 128, H)
            .transpose(1, 0, 2).reshape(128, K * 2 * H)
        ).astype(f16),
        "w_bot": np.ascontiguousarray(
            (W_h[:, H:, :] * (WBOT_SCALE / A)).reshape(K * 2, 128, H)
            .transpose(1, 0, 2).reshape(128, K * 2 * H)
        ).astype(f16),
        "b_h": np.ascontiguousarray(
            np.asarray(b_h, np.float32).reshape(K, 2, 128)
            .transpose(2, 0, 1).reshape(128, K * 2)
        ),
        "w_dec": np.ascontiguousarray(
            np.asarray(W_dec, np.float32).reshape(2, 128, DOUT)
            .transpose(1, 0, 2).reshape(128, 2 * DOUT)
        ).astype(f16),
        "sel": np.ascontiguousarray(
            np.tile(
                np.repeat(np.eye(ST_EX, dtype=np.float32), A, axis=1)
                / WBOT_SCALE,
                (2, 1),
            )
        ).astype(f16),
    }
    in_maps = []
    for i in range(n_cores):
        shard = x[i * bs : (i + 1) * bs].reshape(bs * A, DIN)
        in_maps.append(
            {**common, "xT": np.ascontiguousarray(shard.T).astype(f16)}
        )
    return in_maps


_NC_CACHE = None


def _get_nc():
    global _NC_CACHE
    if _NC_CACHE is None:
        _NC_CACHE = build_nc()
    return _NC_CACHE


def kernel(x, W_enc, b_enc, W_h, b_h, W_dec, b_dec, _run_kwargs=None):
    in_maps = host_inputs(x, W_enc, b_enc, W_h, b_h, W_dec, b_dec)
    nc = _get_nc()
    res = run_bass_kernel_spmd(nc, in_maps, list(range(N_CORES)), **(_run_kwargs or {}))
    b_dec32 = np.asarray(b_dec, np.float32)
    outs = [
        res.results[i]["y"].astype(np.float32).T.reshape(BS, A, DOUT) + b_dec32
        for i in range(N_CORES)
    ]
    full = np.concatenate(outs, axis=0)
    if _run_kwargs:
        kernel.last_results = res
    return full


# revision 21
# speedup vs baseline: 1.0666x; 1.0666x over previous
"""CommNet (B=4096, A=50, DIN=128, H=256, DOUT=64, K=2) on 8 TRN2 NeuronCores.

Data-parallel over batch: 512 examples (25600 agent-tokens) per core, weights
replicated, feature-major ([feature, token]) on-chip layout with the host
pre-transposing each x shard (fp16 - halves input DMA bytes).

Final design (266 us HW exec, rel err 6.1e-4; baseline was 286 us):
- All dense matmuls fp16 at N=400 (one 8-example sub-tile per PSUM bank);
  N=400 streams at 1 col/cycle back-to-back, where N=512 measured 0.61 ns/col.
- tanh ACTIVATEs cover whole 3-bank PSUM groups ({3,3,2} sub-tiles) so the
  ScalarE ~172-cycle per-op overhead amortizes; groups are emitted
  m-interleaved so the 2-slot PSUM ring never backs up into the PE queue.
- Agent-mean path: S = per-example sums (one DVE reduce per [128,3200] half),
  pcw = (512/A*W_bot)^T S on the PE duplicated into both partition halves by
  column tiling, u16 = fp16(pcw) on DVE, then broadcast back over agents by
  ROW-TILED pairs of fp16 matmuls against a constant 0/2^-9 selector - the
  two members of a pair contract in rows 0-63 / 64-127 of the PE array and
  run concurrently (145 ns/matmul vs 224 serial).
- Decoder (M=64) is COLUMN-TILED: pairs of sub-tiles run concurrently in the
  two 64-column halves of the array (133 ns avg), outputs land in the two
  partition halves of shared banks, copied to fp16 and DMA'd as strided
  even/odd token blocks. Decoder bias is folded into the host-side unshard.
- Schedule: 2 groups of 4 supertiles; cw matmuls staggered one supertile
  behind their producing reduce, decoders staggered into the last comm pass
  (DVE is idle there), and the next group's x DMAs prefetched a full group
  early - each fix removes a measured multi-us in-order engine-queue stall.
"""

import numpy as np

import concourse.bacc as bacc
import concourse.bass as bass
import concourse.tile as tile
from concourse import mybir
from concourse.bass_utils import run_bass_kernel_spmd

N_CORES = 8
B, A, DIN, H, DOUT, K = 4096, 50, 128, 256, 64, 2
BS = B // N_CORES          # examples per core
TOK = BS * A               # tokens per core
ST_EX = 64                 # examples per supertile
ST = ST_EX * A             # 3200 tokens per supertile
BANK = 512                 # fp32 elems per PSUM bank
SUB = 400                  # tokens per sub-tile (8 examples; N=400 streams
                           # at 1 col/cycle on HW where N=512 does not)
WBOT_SCALE = 512.0         # host folds 512/A into w_bot; sel carries 2^-9
DEC_COLTILE = True

# (token_lo, [widths]) groups; each group = one 3-bank PSUM tile filled,
# activated, and released before the next group needs its ring slot.
GROUPS = [(0, [400, 400, 400]), (1200, [400, 400, 400]), (2400, [400, 400])]

F32 = mybir.dt.float32
F16 = mybir.dt.float16
Tanh = mybir.ActivationFunctionType.Tanh


def build_nc(n_supertiles=BS // ST_EX, ilv=4):
    tok = n_supertiles * ST
    nc = bacc.Bacc(
        "TRN2",
        target_bir_lowering=False,
        debug=False,
        enable_asserts=True,
        num_devices=N_CORES,
    )
    xT = nc.dram_tensor("xT", [DIN, tok], F16, kind="ExternalInput")
    w_enc = nc.dram_tensor("w_enc", [DIN, H], F16, kind="ExternalInput")
    b_enc = nc.dram_tensor("b_enc", [128, 2], F32, kind="ExternalInput")
    w_top = nc.dram_tensor("w_top", [128, K * 2 * H], F16, kind="ExternalInput")
    w_bot = nc.dram_tensor("w_bot", [128, K * 2 * H], F16, kind="ExternalInput")
    b_h = nc.dram_tensor("b_h", [128, K * 2], F32, kind="ExternalInput")
    w_dec = nc.dram_tensor("w_dec", [128, 2 * DOUT], F16, kind="ExternalInput")
    sel = nc.dram_tensor("sel", [128, ST], F16, kind="ExternalInput")
    y = nc.dram_tensor("y", [DOUT, tok], F16, kind="ExternalOutput")

    with tile.TileContext(nc) as tc:
        with (
            tc.tile_pool(name="wpool", bufs=1) as wpool,
            tc.tile_pool(name="xpool", bufs=2 * ilv) as xpool,
            tc.tile_pool(name="hpool", bufs=ilv) as hpool,
            tc.tile_pool(name="spool", bufs=ilv) as spool,
            tc.tile_pool(name="opool", bufs=2) as opool,
            tc.tile_pool(name="psmm", bufs=2, space=bass.MemorySpace.PSUM) as psmm,
            tc.tile_pool(name="pscw", bufs=2, space=bass.MemorySpace.PSUM) as pscw,
        ):
            wenc_sb = wpool.tile([DIN, H], F16)
            nc.sync.dma_start(wenc_sb[:], w_enc[:])
            benc_sb = wpool.tile([128, 2], F32)
            nc.sync.dma_start(benc_sb[:], b_enc[:])
            wtop_sb = wpool.tile([128, K * 2 * H], F16)
            wbot_sb = wpool.tile([128, K * 2 * H], F16)
            bh_sb = wpool.tile([128, K * 2], F32)
            nc.sync.dma_start(bh_sb[:], b_h[:])
            wdec_sb = wpool.tile([128, 2 * DOUT], F16)
            sel_sb = wpool.tile([128, ST], F16)

            def load_bulk_weights():
                nc.gpsimd.dma_start(wtop_sb[:], w_top[:])
                nc.gpsimd.dma_start(wbot_sb[:], w_bot[:])
                nc.gpsimd.dma_start(wdec_sb[:], w_dec[:])
                nc.gpsimd.dma_start(sel_sb[:], sel[:])

            def make_state(s):
                xt = xpool.tile([DIN, ST], F16, tag="xt", name=f"xt_{s}")
                for c0 in range(0, ST, 800):
                    nc.gpsimd.dma_start(
                        xt[:, c0 : c0 + 800],
                        xT[:, s * ST + c0 : s * ST + c0 + 800],
                    )
                hA = hpool.tile([128, 2, ST], F16, tag="hA", name=f"hA_{s}")
                hB = hpool.tile([128, 2, ST], F16, tag="hB", name=f"hB_{s}")
                S_t = [
                    spool.tile([128, 2, ST_EX], F16, tag=f"S{k}", name=f"S{k}_{s}")
                    for k in range(K)
                ]
                u_t = [
                    spool.tile([128, H], F16, tag=f"u{k}", name=f"u{k}_{s}")
                    for k in range(K)
                ]
                return {"s": s, "xt": xt, "hA": hA, "hB": hB, "S": S_t, "u": u_t}

            def reduce_S(st, h, k, m):
                seg = h[:, m, :].rearrange("p (e a) -> p e a", a=A)
                with nc.allow_low_precision(reason="fp16 agent-sum"):
                    nc.vector.reduce_sum(
                        st["S"][k][:, m, :], seg, axis=mybir.AxisListType.X
                    )

            def cw_phase(st, k):
                s = st["s"]
                pcw = pscw.tile([128, BANK], F32, tag="pcw", name=f"pcw_{s}_{k}")
                for kc in range(2):
                    off = (k * 2 + kc) * H
                    for half in range(2):
                        p0 = half * ST_EX
                        nc.tensor.matmul(
                            pcw[p0 : p0 + ST_EX, 0:H],
                            st["S"][k][:, kc, :],
                            wbot_sb[:, off : off + H],
                            start=(kc == 0),
                            stop=(kc == 1),
                            skip_group_check=True,
                        )
                with nc.allow_low_precision(reason="fp16 u (pcw is 512x scaled)"):
                    nc.vector.tensor_copy(st["u"][k][:], pcw[:, 0:H])

            def act_group(ps, h_out, m, lo, widths, bias):
                span = sum(widths)
                n = len(widths)
                w = widths[0]
                src = ps[:].rearrange("p (g b) -> p g b", b=BANK)[:, 0:n, 0:w]
                dst = h_out[:, m, lo : lo + span].rearrange(
                    "p (g b) -> p g b", b=w
                )
                nc.scalar.activation(dst, src, Tanh, bias=bias)

            def enc_phase(st):
                s, xt, hA = st["s"], st["xt"], st["hA"]
                for lo, widths in GROUPS:
                    for m in range(2):
                        ps = psmm.tile(
                            [128, 3 * BANK], F32, tag="ps",
                            name=f"pse_{s}_{m}_{lo}",
                        )
                        for j, w in enumerate(widths):
                            t0 = lo + j * SUB
                            nc.tensor.matmul(
                                ps[:, j * BANK : j * BANK + w],
                                wenc_sb[:, m * 128 : (m + 1) * 128],
                                xt[:, t0 : t0 + w],
                                start=True,
                                stop=True,
                            )
                        act_group(ps, hA, m, lo, widths, benc_sb[:, m : m + 1])
                        if lo == GROUPS[-1][0]:
                            reduce_S(st, hA, 0, m)
                cw_phase(st, 0)

            def comm_phase(st, k):
                s = st["s"]
                hcur = st["hA"] if k == 0 else st["hB"]
                hnxt = st["hB"] if k == 0 else st["hA"]
                u16 = st["u"][k]
                for lo, widths in GROUPS:
                    for m in range(2):
                        n = len(widths)
                        ps = psmm.tile(
                            [128, 3 * BANK], F32, tag="ps",
                            name=f"psc_{s}_{k}_{lo}_{m}",
                        )
                        for kc in range(2):
                            off = (k * 2 + kc) * H + m * 128
                            for j, w in enumerate(widths):
                                t0 = lo + j * SUB
                                nc.tensor.matmul(
                                    ps[:, j * BANK : j * BANK + w],
                                    wtop_sb[:, off : off + 128],
                                    hcur[:, kc, t0 : t0 + w],
                                    start=(kc == 0),
                                    stop=False,
                                )
                        j = 0
                        while j < n:
                            for half in range(1 + (j + 1 < n)):
                                jj = j + half
                                p0 = half * ST_EX
                                t0 = lo + jj * SUB
                                nc.tensor.matmul(
                                    ps[:, jj * BANK : jj * BANK + widths[jj]],
                                    u16[p0 : p0 + ST_EX,
                                        m * 128 : (m + 1) * 128],
                                    sel_sb[p0 : p0 + ST_EX,
                                           t0 : t0 + widths[jj]],
                                    start=False,
                                    stop=True,
                                    skip_group_check=True,
                                )
                            j += 2
                        act_group(
                            ps, hnxt, m, lo, widths,
                            bh_sb[:, k * 2 + m : k * 2 + m + 1],
                        )
                        if k + 1 < K and lo == GROUPS[-1][0]:
                            reduce_S(st, hnxt, k + 1, m)

            def dec_phase(st):
                s = st["s"]
                hcur = st["hA"] if K % 2 == 0 else st["hB"]
                # 8 sub-tiles as 4 column-tiled pairs: the two members of a
                # pair run concurrently in the two 64-column halves of the
                # PE array (out partitions 0-63 / 64-127), one bank each.
                out_t = opool.tile([128, 4 * SUB], F16, tag="out",
                                   name=f"out_{s}")
                ps = psmm.tile([128, 3 * BANK], F32, tag="ps", name=f"psd_{s}")
                ps2 = psmm.tile([128, 3 * BANK], F32, tag="ps", name=f"psd2_{s}")
                for pair in range(4):
                    pt, bank = (ps, pair) if pair < 3 else (ps2, 0)
                    for kc in range(2):
                        wslice = wdec_sb[:, kc * DOUT : (kc + 1) * DOUT]
                        for half in range(2):
                            t0 = (2 * pair + half) * SUB
                            p0 = half * DOUT
                            nc.tensor.matmul(
                                pt[p0 : p0 + DOUT,
                                   bank * BANK : bank * BANK + SUB],
                                wslice,
                                hcur[:, kc, t0 : t0 + SUB],
                                start=(kc == 0),
                                stop=(kc == 1),
                                skip_group_check=True,
                            )
                    with nc.allow_low_precision(reason="fp16 output"):
                        for half in range(2):
                            p0, p1 = half * DOUT, (half + 1) * DOUT
                            nc.vector.tensor_copy(
                                out_t[p0:p1, pair * SUB : (pair + 1) * SUB],
                                pt[p0:p1, bank * BANK : bank * BANK + SUB],
                            )
                base = s * ST
                for half in range(2):
                    p0, p1 = half * DOUT, (half + 1) * DOUT
                    yv = y[:, base : base + ST].rearrange(
                        "p (g b) -> p g b", b=2 * SUB
                    )[:, :, half * SUB : (half + 1) * SUB]
                    sv = out_t[p0:p1, 0:1600].rearrange(
                        "p (g b) -> p g b", b=SUB
                    )
                    nc.sync.dma_start(yv, sv)

            assert n_supertiles % ilv == 0 or n_supertiles < ilv
            step = min(ilv, n_supertiles)
            sgroups = [
                list(range(s0, s0 + step))
                for s0 in range(0, n_supertiles, step)
            ]
            sts = [make_state(s) for s in sgroups[0]]
            load_bulk_weights()
            for st in sts:
                enc_phase(st)
            nxt = [make_state(s) for s in sgroups[1]] if len(sgroups) > 1 else None
            for gi, grp in enumerate(sgroups):
                for k in range(K):
                    for i, st in enumerate(sts):
                        comm_phase(st, k)
                        if k + 1 < K and i >= 1:
                            cw_phase(sts[i - 1], k + 1)
                        if k == K - 1 and i >= 1:
                            # stagger decoders into the last comm pass: their
                            # DVE copies land while DVE is otherwise idle and
                            # the next comm phase fills any PE wait.
                            dec_phase(sts[i - 1])
                    if k + 1 < K:
                        cw_phase(sts[-1], k + 1)
                dec_phase(sts[-1])
                if nxt is not None:
                    for nst in nxt:
                        enc_phase(nst)
                    sts = nxt
                    nxt = (
                        [make_state(s) for s in sgroups[gi + 2]]
                        if gi + 2 < len(sgroups)
                        else None
                    )

    nc.compile()
    return nc


def host_inputs(x, W_enc, b_enc, W_h, b_h, W_dec, b_dec, n_cores=N_CORES, bs=BS):
    """Shard x over cores (pre-transposed, fp16); replicate weights."""
    x = np.asarray(x, np.float32)
    f16 = np.float16
    W_h = np.asarray(W_h, np.float32)
    common = {
        "w_enc": np.ascontiguousarray(np.asarray(W_enc, np.float32)).astype(f16),
        "b_enc": np.ascontiguousarray(
            np.asarray(b_enc, np.float32).reshape(2, 128).T
        ),
        "w_top": np.ascontiguousarray(
            W_h[:, :H, :].reshape(K * 2, 128, H)
            .transpose(1, 0, 2).reshape(128, K * 2 * H)
        ).astype(f16),
        "w_bot": np.ascontiguousarray(
            (W_h[:, H:, :] * (WBOT_SCALE / A)).reshape(K * 2, 128, H)
            .transpose(1, 0, 2).reshape(128, K * 2 * H)
        ).astype(f16),
        "b_h": np.ascontiguousarray(
            np.asarray(b_h, np.float32).reshape(K, 2, 128)
            .transpose(2, 0, 1).reshape(128, K * 2)
        ),
        "w_dec": np.ascontiguousarray(
            np.asarray(W_dec, np.float32).reshape(2, 128, DOUT)
            .transpose(1, 0, 2).reshape(128, 2 * DOUT)
        ).astype(f16),
        "sel": np.ascontiguousarray(
            np.tile(
                np.repeat(np.eye(ST_EX, dtype=np.float32), A, axis=1)
                / WBOT_SCALE,
                (2, 1),
            )
        ).astype(f16),
    }
    in_maps = []
    for i in range(n_cores):
        shard = x[i * bs : (i + 1) * bs].reshape(bs * A, DIN)
        in_maps.append(
            {**common, "xT": np.ascontiguousarray(shard.T).astype(f16)}
        )
    return in_maps


_NC_CACHE = None


def _get_nc():
    global _NC_CACHE
    if _NC_CACHE is None:
        _NC_CACHE = build_nc()
    return _NC_CACHE


def kernel(x, W_enc, b_enc, W_h, b_h, W_dec, b_dec, _run_kwargs=None):
    in_maps = host_inputs(x, W_enc, b_enc, W_h, b_h, W_dec, b_dec)
    nc = _get_nc()
    res = run_bass_kernel_spmd(nc, in_maps, list(range(N_CORES)), **(_run_kwargs or {}))
    b_dec32 = np.asarray(b_dec, np.float32)
    outs = [
        res.results[i]["y"].astype(np.float32).T.reshape(BS, A, DOUT) + b_dec32
        for i in range(N_CORES)
    ]
    full = np.concatenate(outs, axis=0)
    if _run_kwargs:
        kernel.last_results = res
    return full
